# revision 3
# baseline (speedup 1.0000x reference)
"""Capsule dynamic-routing kernel for 8 Trainium2 NeuronCores — bf16 redesign.

Sharding: input-capsule dim IN_N=2048 split across 8 cores (256 each), full
batch B=128 per core; per-round partial sums s [B,32,16] AllReduced. W split 8x.

vs the fp32 baseline:
- bf16 datapath: 2x DVE modes for the two big elementwise passes, bf16 PE
  operands (FWL weight loads), halved DMA.
- ACT casts V (PSUM f32 -> SBUF bf16) so DVE multiplies run at 2x.
- softmax denominator accumulated online per-j on GPSIMD; 1/den folded into
  x once per round (xz), so the per-j y-multiply is a single 2x TT.
- (i,e) axis split into halves h: B(h1) overlaps D(h0) via interleaved
  emission; softmax factorizes per half.
- y transposed in bf16 (bf16 PSUM tiles), PSUM->SBUF copies split ACT/DVE.
- GPSIMD takes 1-in-4 of the elementwise multiplies plus logit adds.
"""

import os
from contextlib import ExitStack

import numpy as np

import concourse.bass as bass
import concourse.bacc as bacc
import concourse.tile as tile
from concourse import mybir, masks
from concourse.bass_utils import run_bass_kernel_spmd

B, IN_N, IN_D = 128, 2048, 8
CAPS, DIM = 32, 16
JD = CAPS * DIM          # 512
N_CORES = 8
I_LOC = IN_N // N_CORES  # 256
NQ = I_LOC // 16         # 16 chunks of 16 i's
IE = I_LOC * IN_D        # 2048 (i,e) per core
HIE = IE // 2            # 1024 per i-half
EPS = 1e-7
F32 = mybir.dt.float32
BF16 = mybir.dt.bfloat16

GP_MULT = int(os.environ.get("CAPS_GP_MULT", "4"))   # every GP_MULT-th mult on gpsimd (0=off)
YT_ACT = int(os.environ.get("CAPS_YT_ACT", "2"))     # 1-in-YT_ACT yt copies on ACT (rest DVE)
GP_ADD = int(os.environ.get("CAPS_GP_ADD", "1"))     # den/blog adds on gpsimd (0=DVE)
NO_ILV = int(os.environ.get("CAPS_NO_ILV", "0"))     # 1 = do not interleave D(h0) with B(h1)
DIRECT_V = int(os.environ.get("CAPS_DIRECT_V", "0"))
AR_BF16 = int(os.environ.get("CAPS_AR_BF16", "1"))    # 1 = AllReduce payloads in bf16  # 1-in-N V-mults read PSUM directly on DVE (0=off)

_CACHE = {}
LAST_RESULT = None


def _prep_core_inputs(x, W, k):
    sl = slice(k * I_LOC, (k + 1) * I_LOC)
    Wk = W[:, sl]                                   # [32, 256, 16, 8]
    xk = x[:, sl]                                   # [128, 256, 8]
    Wr = np.ascontiguousarray(Wk.transpose(1, 3, 0, 2)).reshape(I_LOC, IN_D, JD)
    xT = np.ascontiguousarray(xk.transpose(1, 2, 0))          # [i, e, b]

    # r0 dense layouts (bf16)
    wr = Wr.reshape(NQ, 16 * IN_D, JD)
    wr = np.ascontiguousarray(wr.transpose(1, 0, 2)).reshape(128, NQ * JD)
    xt = xT.reshape(NQ, 16 * IN_D, B)
    xt = np.ascontiguousarray(xt.transpose(1, 0, 2)).reshape(128, NQ * B)

    # x in [b, (i,e)] (i-major) and [b, (e,i)] (e-major)
    xbie = np.ascontiguousarray(xk).reshape(B, IE)
    xe = np.ascontiguousarray(xk.transpose(0, 2, 1)).reshape(B, IE)

    # V-mm moving: wdq[beta, h, d, m*HIE + ii*8 + e] = W[2*beta+m, 128*h+ii, d, e]
    Wd = Wk.transpose(0, 2, 1, 3).reshape(CAPS, DIM, 2, 128 * IN_D)  # [j, d, h, (ii,e)]
    wdq = np.ascontiguousarray(
        Wd.reshape(16, 2, DIM, 2, 128 * IN_D).transpose(0, 3, 2, 1, 4)
    ).reshape(16, 2, DIM, 2 * 128 * IN_D)

    # s-mm stationary: wsd2[p, j*256 + q*16 + d] = Wk[j, 128*(q%2)+p, d, q//2]
    t = Wk.transpose(1, 0, 3, 2).reshape(2, 128, CAPS, IN_D, DIM)  # [ih, p, j, e, d]
    wsd2 = np.ascontiguousarray(t.transpose(1, 2, 3, 0, 4)).reshape(128, CAPS * NQ * DIM)

    def bf(a):
        import jax.numpy as jnp
        return np.asarray(jnp.asarray(a, dtype=jnp.bfloat16))

    return {"wr": bf(wr), "xt": bf(xt), "xbie": bf(xbie), "xe": bf(xe),
            "wdq": bf(wdq), "wsd2": bf(wsd2)}


def _squash(nc, pool, s_ap, scale, obuf):
    sqt = pool.tile([128, JD], F32, tag="sq_t")
    nc.scalar.activation(sqt[:], s_ap, mybir.ActivationFunctionType.Square,
                         scale=float(scale))
    sq = pool.tile([128, CAPS], F32, tag="sq")
    nc.vector.tensor_reduce(sq[:], sqt[:].rearrange("p (j d) -> p j d", d=DIM),
                            axis=mybir.AxisListType.X, op=mybir.AluOpType.add)
    t1 = pool.tile([128, CAPS], F32, tag="sqa")
    nc.vector.tensor_scalar_add(t1[:], sq[:], 1.0)
    sqe = pool.tile([128, CAPS], F32, tag="sqf")
    nc.vector.tensor_scalar_add(sqe[:], sq[:], EPS)
    rt = pool.tile([128, CAPS], F32, tag="sqb")
    nc.scalar.activation(rt[:], sqe[:], mybir.ActivationFunctionType.Sqrt)
    den = pool.tile([128, CAPS], F32, tag="sqc")
    nc.vector.tensor_mul(den[:], t1[:], rt[:])
    rden = pool.tile([128, CAPS], F32, tag="sqd")
    nc.vector.reciprocal(rden[:], den[:])
    fac = pool.tile([128, CAPS], F32, tag="sqe")
    nc.vector.tensor_mul(fac[:], sq[:], rden[:])
    if scale != 1.0:
        nc.vector.tensor_scalar_mul(fac[:], fac[:], float(scale))
    nc.vector.tensor_mul(
        obuf[:].rearrange("p (j d) -> p j d", d=DIM),
        s_ap.rearrange("p (j d) -> p j d", d=DIM),
        fac[:].unsqueeze(-1).broadcast_to([128, CAPS, DIM]))


def _build(num_cores, reps=1):
    nc = bacc.Bacc("TRN2", target_bir_lowering=False, debug=False,
                   num_devices=num_cores)
    group = [list(range(num_cores))]

    wr_d = nc.dram_tensor("wr", [128, NQ * JD], BF16, kind="ExternalInput")
    xt_d = nc.dram_tensor("xt", [128, NQ * B], BF16, kind="ExternalInput")
    xbie_d = nc.dram_tensor("xbie", [128, IE], BF16, kind="ExternalInput")
    xe_d = nc.dram_tensor("xe", [128, IE], BF16, kind="ExternalInput")
    wdq_d = nc.dram_tensor("wdq", [16, 2, DIM, 2 * HIE], BF16, kind="ExternalInput")
    wsd2_d = nc.dram_tensor("wsd2", [128, CAPS * NQ * DIM], BF16, kind="ExternalInput")
    out_d = nc.dram_tensor("out", [128, JD], F32, kind="ExternalOutput")

    with tile.TileContext(nc) as tc, ExitStack() as ctx:
        const = ctx.enter_context(tc.tile_pool(name="const", bufs=1))
        idp = ctx.enter_context(tc.tile_pool(name="idp", bufs=1))
        sm = ctx.enter_context(tc.tile_pool(name="small", bufs=2))
        wrp = ctx.enter_context(tc.tile_pool(name="wrp", bufs=3))
        wdp = ctx.enter_context(tc.tile_pool(name="wdp", bufs=3))
        vbp = ctx.enter_context(tc.tile_pool(name="vbp", bufs=3))
        prp = ctx.enter_context(tc.tile_pool(name="prp", bufs=3))
        agp = ctx.enter_context(tc.tile_pool(name="agp", bufs=2))
        yjp = ctx.enter_context(tc.tile_pool(name="yjp", bufs=2))
        ytp = ctx.enter_context(tc.tile_pool(name="ytp", bufs=4))
        pvp = ctx.enter_context(tc.tile_pool(name="pvp", bufs=1, space="PSUM"))
        psp = ctx.enter_context(tc.tile_pool(name="psp", bufs=2, space="PSUM"))
        stp = ctx.enter_context(tc.tile_pool(name="stp", bufs=1, space="PSUM"))
        dram = ctx.enter_context(tc.tile_pool(name="dram", bufs=1, space="DRAM"))

        ident = idp.tile([128, 128], F32)
        masks.make_identity(nc, ident[:])
        identb = idp.tile([128, 128], BF16)
        masks.make_identity(nc, identb[:])

        mul_ix = [0]

        def mult(out, a, b2):
            eng = nc.gpsimd if (GP_MULT and mul_ix[0] % GP_MULT == GP_MULT - 1) \
                else nc.vector
            mul_ix[0] += 1
            eng.tensor_mul(out, a, b2)

        for _rep in range(reps):
            xt_s = const.tile([128, NQ * B], BF16, tag="xt", bufs=2)
            nc.sync.dma_start(xt_s[:], xt_d.ap())
            xbie_s = const.tile([128, IE], BF16, tag="xbie", bufs=2)
            nc.sync.dma_start(xbie_s[:], xbie_d.ap())
            xe_s = const.tile([128, IE], BF16, tag="xe", bufs=2)
            nc.sync.dma_start(xe_s[:], xe_d.ap())
            wsd_s = const.tile([128, CAPS * NQ * DIM], BF16, tag="wsd", bufs=2)
            nc.sync.dma_start(wsd_s[:], wsd2_d.ap())

            blog = const.tile([128, CAPS * I_LOC], F32, tag="blog")   # [b,(j,i)]
            cexp = const.tile([128, CAPS * I_LOC], BF16, tag="cexp")  # [b,(j,i)]
            xz = const.tile([128, IE], BF16, tag="xz")                # [b,(e,i)]
            oTall = const.tile([16, CAPS * B], BF16, tag="oTall")     # [d,(j,b)]
            obuf = const.tile([128, JD], F32, tag="obuf")
            ART = BF16 if AR_BF16 else F32
            s_sb = const.tile([128, JD], ART, tag="s_sb")

            # ---------------- round 0: merged matmul ----------------
            # ps0 shares the sT slot (never live at the same time as sT)
            ps0 = stp.tile([128, JD], F32, tag="sT", bufs=1, name=f"ps0_{_rep}")
            for c in range(NQ):
                wr_t = wrp.tile([128, JD], BF16, tag="wr")
                nc.sync.dma_start(wr_t[:], wr_d.ap()[:, c * JD:(c + 1) * JD])
                nc.tensor.matmul(ps0[:], xt_s[:, c * B:(c + 1) * B], wr_t[:],
                                 start=(c == 0), stop=(c == NQ - 1))

            def allreduce_s(src_ap, scale, rnd):
                inb = dram.tile([128, JD], ART, tag=f"arin{rnd}")
                outb = dram.tile([128, JD], ART, tag=f"arout{rnd}",
                                 addr_space="Shared")
                nc.sync.dma_start(inb[:], src_ap)
                nc.gpsimd.collective_compute(
                    "AllReduce", mybir.AluOpType.add, replica_groups=group,
                    ins=[inb[:].opt()], outs=[outb[:].opt()])
                sf = sm.tile([128, JD], ART, tag="sfull")
                nc.sync.dma_start(sf[:], outb[:])
                _squash(nc, sm, sf[:], scale, obuf)

            s0s = sm.tile([128, JD], ART, tag="s0s")
            nc.scalar.copy(s0s[:], ps0[:])
            allreduce_s(s0s[:], 1.0 / CAPS, 0)

            # ---------------- rounds 1, 2 ----------------
            for rnd in (1, 2):
                den = [None, None]
                rden = [None, None]
                for h in range(2):
                    den[h] = sm.tile([128, 128], F32, tag=f"den{h}", bufs=1,
                                     name=f"den{h}_{rnd}_{_rep}")
                    nc.vector.memset(den[h][:], 0.0)

                sT = stp.tile([128, 2048], F32, tag="sT", bufs=1,
                              name=f"sT_{rnd}_{_rep}")

                def bstep(j, h):
                    """V-mm -> cast -> mult -> reduce -> (add) -> exp -> den."""
                    beta, m = j // 2, j % 2
                    if m == 0:
                        wd_t = wdp.tile([DIM, 2 * HIE], BF16, tag=f"wd{h}")
                        nc.sync.dma_start(wd_t[:], wdq_d.ap()[beta, h])
                        bstep.wd = wd_t
                    wd_t = bstep.wd
                    direct = DIRECT_V and (2 * j + h) % DIRECT_V == 0
                    vb = None if direct else vbp.tile([128, HIE], BF16, tag="vb")
                    prod = prp.tile([128, HIE], BF16, tag="prod")
                    for n2 in range(2):
                        pv = pvp.tile([128, 512], F32, tag=f"pv{n2}")
                        nc.tensor.matmul(
                            pv[:],
                            oTall[:, j * B:(j + 1) * B],
                            wd_t[:, m * HIE + n2 * 512:m * HIE + (n2 + 1) * 512],
                            start=True, stop=True)
                        if direct:
                            # DVE reads PSUM f32 directly (1x) - skips ACT cast
                            nc.vector.tensor_mul(
                                prod[:, n2 * 512:(n2 + 1) * 512], pv[:],
                                xbie_s[:, h * HIE + n2 * 512:
                                       h * HIE + (n2 + 1) * 512])
                        else:
                            nc.scalar.copy(vb[:, n2 * 512:(n2 + 1) * 512], pv[:])
                    if not direct:
                        # xbie is i-major: half h occupies cols [h*HIE, (h+1)*HIE)
                        mult(prod[:], vb[:], xbie_s[:, h * HIE:(h + 1) * HIE])
                    dst = blog[:, j * I_LOC + h * 128: j * I_LOC + h * 128 + 128]
                    if rnd == 1:
                        nc.vector.tensor_reduce(
                            dst, prod[:].rearrange("p (i e) -> p i e", e=IN_D),
                            axis=mybir.AxisListType.X, op=mybir.AluOpType.add)
                    else:
                        ag = agp.tile([128, 128], F32, tag="ag")
                        nc.vector.tensor_reduce(
                            ag[:], prod[:].rearrange("p (i e) -> p i e", e=IN_D),
                            axis=mybir.AxisListType.X, op=mybir.AluOpType.add)
                        blog_eng = nc.gpsimd if (
                            GP_ADD in (1, 3) or (GP_ADD == 2 and j % 2 == 0)
                        ) else nc.vector
                        blog_eng.tensor_add(dst, dst, ag[:])
                    ce = cexp[:, j * I_LOC + h * 128: j * I_LOC + h * 128 + 128]
                    nc.scalar.activation(ce, dst, mybir.ActivationFunctionType.Exp)
                    den_eng = nc.gpsimd if (
                        GP_ADD == 1 or (GP_ADD == 2 and h == 0)
                    ) else nc.vector  # GP_ADD=3: den on DVE, blog on GP
                    den_eng.tensor_add(den[h][:], den[h][:], ce)

                def cstep(h):
                    rden[h] = sm.tile([128, 128], F32, tag=f"rden{h}", bufs=1,
                                      name=f"rden{h}_{rnd}_{_rep}")
                    nc.vector.reciprocal(rden[h][:], den[h][:])
                    xzv = xz[:].rearrange("p (e i) -> p e i", i=I_LOC)[
                        :, :, h * 128:(h + 1) * 128]
                    xev = xe_s[:].rearrange("p (e i) -> p e i", i=I_LOC)[
                        :, :, h * 128:(h + 1) * 128]
                    nc.vector.tensor_mul(
                        xzv, xev,
                        rden[h][:].unsqueeze(1).broadcast_to([128, IN_D, 128]))

                def dstep(j, h):
                    """y-mult -> transposes -> yt copies -> s-mm accumulate."""
                    yj = yjp.tile([128, HIE], BF16, tag="yj")
                    xzv = xz[:].rearrange("p (e i) -> p e i", i=I_LOC)[
                        :, :, h * 128:(h + 1) * 128]
                    cv = cexp[:, j * I_LOC + h * 128: j * I_LOC + h * 128 + 128]
                    mult(yj[:].rearrange("p (e i) -> p e i", i=128),
                         cv.unsqueeze(1).broadcast_to([128, IN_D, 128]), xzv)
                    grp, mj = j // 4, j % 4
                    yts = []
                    for q4 in range(2):
                        pt = psp.tile([128, 512], BF16, tag="pst")
                        for t in range(4):
                            u = 4 * q4 + t
                            nc.tensor.transpose(
                                pt[:, 128 * t:128 * (t + 1)],
                                yj[:, 128 * u:128 * (u + 1)], identb[:])
                        yt = ytp.tile([128, 512], BF16, tag="yt")
                        if YT_ACT and (2 * j + q4) % YT_ACT == 0:
                            nc.scalar.copy(yt[:], pt[:])
                        else:
                            nc.vector.tensor_copy(yt[:], pt[:])
                        yts.append(yt)
                    # 8 consecutive accumulating matmuls into this (j,h) region
                    for u in range(8):
                        q = 2 * u + h
                        nc.tensor.matmul(
                            sT[32 * mj:32 * mj + DIM,
                               256 * grp + 128 * h:256 * grp + 128 * h + 128],
                            wsd_s[:, j * (NQ * DIM) + q * DIM:
                                  j * (NQ * DIM) + (q + 1) * DIM],
                            yts[u // 4][:, 128 * (u % 4):128 * (u % 4 + 1)],
                            start=(u == 0), stop=(u == 7),
                            tile_position=(0, 32 * mj))

                # oT(j) + B(j, h=0), pipelined per j
                for j in range(CAPS):
                    pto = psp.tile([128, 512], F32, tag="pst")
                    nc.tensor.transpose(pto[:DIM, :128],
                                        obuf[:, DIM * j:DIM * (j + 1)], ident[:])
                    nc.scalar.copy(oTall[:, j * B:(j + 1) * B], pto[:DIM, :128])
                    bstep(j, 0)
                cstep(0)
                # D(h=0) interleaved with B(h=1)
                if NO_ILV:
                    for j in range(CAPS):
                        bstep(j, 1)
                    for j in range(CAPS):
                        dstep(j, 0)
                else:
                    for j in range(CAPS):
                        dstep(j, 0)
                        bstep(j, 1)
                cstep(1)

                def estep(grp):
                    # E: sum h-halves of sT, transpose to [b,(j,d)] for this grp
                    st0 = sm.tile([128, 128], F32, tag="st0")
                    nc.scalar.copy(st0[:], sT[:, 256 * grp:256 * grp + 128])
                    stt = sm.tile([128, 128], F32, tag="stt")
                    nc.vector.tensor_add(stt[:], st0[:],
                                         sT[:, 256 * grp + 128:256 * grp + 256])
                    ptb = psp.tile([128, 512], F32, tag="pst")
                    nc.tensor.transpose(ptb[:, :128], stt[:], ident[:])
                    src = ptb[:, :128].rearrange("p (m t) -> p m t", m=4)[:, :, :16]
                    nc.scalar.copy(
                        s_sb[:].rearrange("p (g m d) -> p g m d", g=8, d=16)[:, grp],
                        src)

                # D(h=1) with E(grp) pipelined in as each 4-j group completes
                for j in range(CAPS):
                    dstep(j, 1)
                    if j % 4 == 3:
                        estep(j // 4)

                allreduce_s(s_sb[:], 1.0, rnd)
                if rnd == 2:
                    nc.sync.dma_start(out_d.ap(), obuf[:])

    nc.compile()
    return nc


def kernel(x, W):
    global LAST_RESULT
    x = np.asarray(x, dtype=np.float32)
    W = np.asarray(W, dtype=np.float32)
    if "nc" not in _CACHE:
        _CACHE["nc"] = _build(N_CORES)
    nc = _CACHE["nc"]
    in_maps = [_prep_core_inputs(x, W, k) for k in range(N_CORES)]
    res = run_bass_kernel_spmd(nc, in_maps, list(range(N_CORES)),
                               trace=bool(os.environ.get("CAPS_TRACE")))
    LAST_RESULT = res
    out = res.results[0]["out"]
    return out.reshape(B, CAPS, DIM).astype(np.float32)
